# revision 1
# baseline (speedup 1.0000x reference)
"""Self-contained Trainium2 Bass kernel: 16-head self-attention (B=4, N=2048,
C=1024, fp32), SPMD across 8 NeuronCores.

Entry point: kernel(**inputs) -> np.ndarray matching the reference module
(qkv projection + scaled-dot-product softmax attention + output projection).
See build_nc() docstring for the kernel design.
"""
import numpy as np

_NC_CACHE = {}


def kernel(x, Wqkv, bqkv, Wproj, bproj):
    from concourse.bass_utils import run_bass_kernel_spmd
    x = np.asarray(x, dtype=np.float32)
    Wqkv = np.asarray(Wqkv, dtype=np.float32)
    bqkv = np.asarray(bqkv, dtype=np.float32)
    Wproj = np.asarray(Wproj, dtype=np.float32)
    bproj = np.asarray(bproj, dtype=np.float32)
    # the rank-1 bias accumulation steps are emitted only when any bias is
    # actually nonzero (they are exact zeros in this problem's inputs)
    wb = bool(np.any(bqkv) or np.any(bproj))
    if wb not in _NC_CACHE:
        nc = build_nc(mm_fast=True, with_biases=wb)
        split_excess_waits(nc)
        _NC_CACHE[wb] = nc
    nc = _NC_CACHE[wb]
    in_maps = shard_inputs(x, Wqkv, bqkv, Wproj, bproj)
    res = run_bass_kernel_spmd(nc, in_maps, core_ids=list(range(N_CORES)))
    return unshard_output(res.results).astype(np.float32)


# ======================================================================
# IR post-pass: this walrus build accepts at most one semaphore wait per
# instruction; overflow waits move onto chained NoOps just before the
# instruction on the same engine queue.
# ======================================================================

# Walrus TPB_CTRL codegen (Drain/NoOp lowering) accepts only 1 sync wait;
# regular engine instructions accept more (tested empirically).
CTRL_OPCODES = {"Drain", "NoOp", "EventSemaphore", "AllEngineBarrier"}

def split_excess_waits(nc, engine_max=1, ctrl_max=1):
    n_split = 0
    for f in nc.m.functions:
        for bb in f.blocks:
            insts = list(bb.instructions)
            out = []
            changed = False
            for inst in insts:
                si = inst.sync_info
                max_w = ctrl_max if inst.opcode in CTRL_OPCODES else engine_max
                if si is not None and si.on_wait and len(si.on_wait) > max_w:
                    waits = list(si.on_wait)
                    extra, keep = waits[max_w:], waits[:max_w]
                    for i in range(0, len(extra), ctrl_max):
                        nop = bass_rust.InstNoOp(
                            name=f"{inst.name}-wsplit{i}", ins=[], outs=[])
                        nop.engine = inst.engine
                        nop.sync_info = mybir.SyncInfo(
                            on_wait=extra[i:i + ctrl_max], on_update=[])
                        out.append(nop)
                        n_split += 1
                    inst.sync_info = mybir.SyncInfo(
                        on_wait=keep, on_update=list(si.on_update))
                    changed = True
                out.append(inst)
            if changed:
                bb.instructions = out
    return n_split


# ======================================================================
# Kernel proper
# ======================================================================
import bass_rust
import concourse.bass as bass
import concourse.tile as tile
import concourse.mybir as mybir


F32 = mybir.dt.float32
F32R = mybir.dt.float32r
BF16 = mybir.dt.bfloat16

N = 2048        # sequence length
C = 1024        # embed dim
HL = 8          # heads handled per core
D = 64          # head dim
SCALE = D ** -0.5
NHALF = N // 2
VS = D + 1      # v columns per head incl. ones column
N_CORES = 8

AFT = mybir.ActivationFunctionType
ALU = mybir.AluOpType


def build_nc(mm_fast=True, with_biases=True):
    MDT = F32R if mm_fast else F32
    nc = bass.Bass("TRN2", target_bir_lowering=False, debug=False,
                   num_devices=N_CORES)
    xt = nc.dram_tensor("xt", [C, N], MDT, kind="ExternalInput").ap()
    wq = nc.dram_tensor("wq", [C, HL * D], MDT, kind="ExternalInput").ap()
    wk = nc.dram_tensor("wk", [C, HL * D], MDT, kind="ExternalInput").ap()
    wv = nc.dram_tensor("wv", [C, HL * D], MDT, kind="ExternalInput").ap()
    wp = nc.dram_tensor("wp", [HL * D, C], MDT, kind="ExternalInput").ap()
    bqc = nc.dram_tensor("bqc", [128, 4], F32, kind="ExternalInput").ap()
    bkc = nc.dram_tensor("bkc", [128, 4], F32, kind="ExternalInput").ap()
    bv = nc.dram_tensor("bv", [1, HL * D], MDT, kind="ExternalInput").ap()
    bp = nc.dram_tensor("bp", [1, C], MDT, kind="ExternalInput").ap()
    ones_row = nc.dram_tensor("ones_row", [1, 512], MDT, kind="ExternalInput").ap()
    out = nc.dram_tensor("out", [N, C], F32, kind="ExternalOutput").ap()

    with tile.TileContext(nc) as tc:
        with tc.tile_pool(name="consts", bufs=1) as consts, \
             tc.tile_pool(name="persist", bufs=1) as persist, \
             tc.tile_pool(name="big", bufs=1) as bigp:

            ones = consts.tile([1, 512], MDT, tag="ones")
            nc.sync.dma_start(out=ones, in_=ones_row)

            # persistent activation tiles (bf16 so the attention matmuls
            # get full 128x128 stationary tiles + fast weight loads):
            #   qT: pair-packed [2 heads' d x 128, pair-group x n]
            #   kTp: per-head [128, head x n] with the partner head's 64
            #        partition rows zeroed (K=128 contraction, zeros kill
            #        the partner-q contribution in the shared qT rhs)
            #   v_sb: per (m-chunk, head) [128, 128]: cols 0:64 v, col 64
            #        ones (softmax denominator), cols 65:128 zero padding
            qT = persist.tile([128, 4 * N], BF16, tag="qT")
            kTp = persist.tile([128, HL * N], BF16, tag="kTp")
            v_sb = persist.tile([128, 16 * HL * 128], BF16, tag="v")
            nc.gpsimd.memset(kTp, 0.0)
            nc.gpsimd.memset(v_sb, 0.0)
            vview = v_sb.rearrange("p (m h e) -> p m h e", h=HL, e=128)
            nc.gpsimd.memset(vview[:, :, :, D:D + 1], 1.0)

            # ---------- Phase A: xT, qT/kT, v (per n-half) ----------
            with tc.tile_pool(name="wpool", bufs=3) as wpool, \
                 tc.tile_pool(name="biases", bufs=1) as biasp, \
                 tc.tile_pool(name="kqp", bufs=2, space="PSUM") as kqp, \
                 tc.tile_pool(name="vpp", bufs=2, space="PSUM") as vpp:
                bqc_sb = biasp.tile([128, 4], F32, tag="bqc")
                bkc_sb = biasp.tile([128, 4], F32, tag="bkc")
                bv_sb = biasp.tile([1, HL * D], MDT, tag="bv")
                nc.sync.dma_start(out=bqc_sb, in_=bqc)
                nc.sync.dma_start(out=bkc_sb, in_=bkc)
                nc.sync.dma_start(out=bv_sb, in_=bv)
                # weight tiles: [128 (C-chunk partition), chunk x cols]
                wq_sb = wpool.tile([128, 8 * 512], MDT, tag="w", name="wq_sb")
                wk_sb = wpool.tile([128, 8 * 512], MDT, tag="w", name="wk_sb")
                wv_sb = wpool.tile([128, 8 * 512], MDT, tag="w", name="wv_sb")
                for c in range(8):
                    nc.sync.dma_start(out=wq_sb[:, c * 512:(c + 1) * 512],
                                      in_=wq[c * 128:(c + 1) * 128, :])
                    nc.sync.dma_start(out=wk_sb[:, c * 512:(c + 1) * 512],
                                      in_=wk[c * 128:(c + 1) * 128, :])
                    nc.sync.dma_start(out=wv_sb[:, c * 512:(c + 1) * 512],
                                      in_=wv[c * 128:(c + 1) * 128, :])

                for nh in range(2):
                    xT = bigp.tile([128, 8 * NHALF], MDT, tag="big",
                                   name=f"xT{nh}")
                    for c in range(8):
                        nc.sync.dma_start(
                            out=xT[:, c * NHALF:(c + 1) * NHALF],
                            in_=xt[c * 128:(c + 1) * 128,
                                   nh * NHALF:(nh + 1) * NHALF])
                    # qT / kT for this half (bias folded into evacuation)
                    for g in range(4):
                        for dstT, w_sb, b_col in ((qT, wq_sb, bqc_sb),
                                                  (kTp, wk_sb, bkc_sb)):
                            for ngl in range(2):
                                ps = kqp.tile([128, 512], F32, tag="kq",
                                              name=f"kq{nh}_{g}_{ngl}")
                                for c in range(8):
                                    nc.tensor.matmul(
                                        ps,
                                        w_sb[:, c * 512 + g * 128:
                                             c * 512 + (g + 1) * 128],
                                        xT[:, c * NHALF + ngl * 512:
                                           c * NHALF + ngl * 512 + 512],
                                        start=(c == 0), stop=(c == 7))
                                n0 = nh * NHALF + ngl * 512
                                if dstT is qT:
                                    nc.vector.tensor_scalar_add(
                                        qT[:, g * N + n0: g * N + n0 + 512],
                                        ps, b_col[:, g:g + 1])
                                else:
                                    for hh in range(2):
                                        h_, r0_ = 2 * g + hh, hh * D
                                        nc.vector.tensor_scalar_add(
                                            kTp[r0_:r0_ + D,
                                                h_ * N + n0: h_ * N + n0 + 512],
                                            ps[r0_:r0_ + D, :],
                                            b_col[r0_:r0_ + D, g:g + 1])
                    # v for this half's m-chunks
                    for ml in range(NHALF // 128):
                        mc = nh * (NHALF // 128) + ml
                        ps = vpp.tile([128, 512], F32, tag="v",
                                      name=f"v{nh}_{ml}")
                        for c in range(8):
                            nc.tensor.matmul(
                                ps,
                                xT[:, c * NHALF + ml * 128:
                                   c * NHALF + (ml + 1) * 128],
                                wv_sb[:, c * 512:(c + 1) * 512],
                                start=(c == 0),
                                stop=(c == 7 and not with_biases))
                        if with_biases:
                            nc.tensor.matmul(ps, ones[0:1, 0:128],
                                             bv_sb[0:1, :],
                                             start=False, stop=True)
                        dst = v_sb[:, mc * HL * 128:(mc + 1) * HL * 128].rearrange(
                            "p (h e) -> p h e", e=128)[:, :, 0:D]
                        nc.vector.tensor_copy(
                            dst, ps.rearrange("p (h e) -> p h e", e=D))

            # attn_outT reuses the xT slot (free after phase A's last read)
            aoT = bigp.tile([128, 4 * N], MDT, tag="big", name="aoT")

            # ---------- Phase B + C: attention, then proj per n-half ----------
            with tc.tile_pool(name="wppool", bufs=1) as wppool, \
                 tc.tile_pool(name="expp", bufs=4) as expp, \
                 tc.tile_pool(name="avsp", bufs=2) as avsp, \
                 tc.tile_pool(name="denp", bufs=2) as denp, \
                 tc.tile_pool(name="bpp", bufs=1) as bpp, \
                 tc.tile_pool(name="pout", bufs=3) as pout:
                wp_sb = wppool.tile([128, 4 * C], MDT, tag="wp", name="wp_sb")
                for g in range(4):
                    nc.sync.dma_start(out=wp_sb[:, g * C:(g + 1) * C],
                                      in_=wp[g * 128:(g + 1) * 128, :])
                bp_sb = bpp.tile([1, C], MDT, tag="bp")
                nc.sync.dma_start(out=bp_sb, in_=bp)

                with tc.tile_pool(name="scp", bufs=2, space="PSUM") as scp, \
                     tc.tile_pool(name="avp", bufs=1, space="PSUM") as avp, \
                     tc.tile_pool(name="pjp", bufs=2, space="PSUM") as pjp:
                    # deferred normalization tails: each unit's divide chain
                    # is emitted after the NEXT unit's m-loop so its DMA/DVE
                    # latency never head-of-line-blocks the in-order PE queue
                    tails = []
                    proj_units = []

                    def emit_proj(nch, jg):
                        ps = pjp.tile([128, 512], F32, tag="pj",
                                      name=f"pj{nch}_{jg}")
                        for g in range(4):
                            nc.tensor.matmul(
                                ps,
                                aoT[:, g * N + nch * 128:
                                    g * N + (nch + 1) * 128],
                                wp_sb[:, g * C + jg * 512:
                                      g * C + jg * 512 + 512],
                                start=(g == 0),
                                stop=(g == 3 and not with_biases))
                        if with_biases:
                            nc.tensor.matmul(
                                ps, ones[0:1, 0:128],
                                bp_sb[0:1, jg * 512:(jg + 1) * 512],
                                start=False, stop=True)
                        po = pout.tile([128, 512], F32, tag="po",
                                       name=f"po{nch}_{jg}")
                        nc.vector.tensor_copy(po, ps)
                        nc.sync.dma_start(
                            out=out[nch * 128:(nch + 1) * 128,
                                    jg * 512:(jg + 1) * 512],
                            in_=po)

                    for nh in range(2):
                        n0 = nh * NHALF
                        for h in range(HL):
                            g, r0 = h // 2, (h % 2) * D
                            av = avp.tile([128, NHALF], F32, tag="av",
                                          name=f"av{h}_{nh}")

                            def av_acc(mcc, ex):
                                for ngl in range(2):
                                    nc.tensor.matmul(
                                        av[:, ngl * 512:(ngl + 1) * 512],
                                        v_sb[:, (mcc * HL + h) * 128:
                                             (mcc * HL + h + 1) * 128],
                                        ex[:, ngl * 512:(ngl + 1) * 512],
                                        start=(mcc == 0), stop=(mcc == 15))

                            # av lags sc/exp by 2 m-chunks so the in-order PE
                            # queue never stalls on an exp still in flight
                            pending = []
                            for mcc in range(16):
                                sc = scp.tile([128, NHALF], F32, tag="sc",
                                              name=f"sc{h}_{nh}_{mcc}")
                                for ngl in range(2):
                                    nc.tensor.matmul(
                                        sc[:, ngl * 512:(ngl + 1) * 512],
                                        kTp[:, h * N + mcc * 128:
                                            h * N + (mcc + 1) * 128],
                                        qT[:, g * N + n0 + ngl * 512:
                                           g * N + n0 + (ngl + 1) * 512],
                                        start=True, stop=True)
                                ex = expp.tile([128, NHALF], BF16, tag="ex",
                                               name=f"ex{h}_{nh}_{mcc}")
                                nc.scalar.activation(ex, sc, AFT.Exp,
                                                     scale=SCALE)
                                pending.append((mcc, ex))
                                if len(pending) > 2:
                                    av_acc(*pending.pop(0))
                            for it in pending:
                                av_acc(*it)
                            avs = avsp.tile([VS, NHALF], MDT, tag="avs",
                                            name=f"avs{h}_{nh}")
                            nc.vector.tensor_copy(avs, av[0:VS, :])
                            # reciprocal of the denominator row: DVE recip
                            # is ~6 cyc/elem along the free dim, so reshape
                            # [1,1024] -> [128,8] via sbuf-to-sbuf DMA, recip
                            # across 128 partitions, and reshape back
                            den = denp.tile([128, NHALF // 128], MDT,
                                            tag="den", name=f"den{h}_{nh}")
                            nc.sync.dma_start(out=den, in_=avs[D:VS, :])
                            rcp = denp.tile([128, NHALF // 128], MDT,
                                            tag="rcp", name=f"rcp{h}_{nh}")
                            with nc.allow_low_precision(reason="softmax denom"):
                                nc.vector.reciprocal(rcp, den)
                            rrow = denp.tile([1, NHALF], MDT, tag="rrow",
                                             name=f"rrow{h}_{nh}")
                            nc.sync.dma_start(out=rrow, in_=rcp)

                            def tail(h=h, nh=nh, n0=n0, g=g, r0=r0,
                                     avs=avs, rrow=rrow):
                                bc = scp.tile([D, NHALF], F32, tag="sc",
                                              name=f"bc{h}_{nh}")
                                for ngl in range(2):
                                    nc.tensor.matmul(
                                        bc[:, ngl * 512:(ngl + 1) * 512],
                                        ones[0:1, 0:D],
                                        rrow[0:1, ngl * 512:(ngl + 1) * 512],
                                        start=True, stop=True)
                                nc.vector.tensor_mul(
                                    aoT[r0:r0 + D,
                                        g * N + n0: g * N + n0 + NHALF],
                                    avs[0:D, :], bc)
                            tails.append(tail)
                            if len(tails) > 1:
                                tails.pop(0)()
                        while tails:
                            tails.pop(0)()
                        for nl in range(8):
                            for jg in range(2):
                                emit_proj(nh * 8 + nl, jg)
    return nc


def shard_inputs(x, Wqkv, bqkv, Wproj, bproj):
    """Full inputs -> per-core in_maps. Core c: batch c//2, head-group c%2."""
    in_maps = []
    for core in range(N_CORES):
        b, hg = core // 2, core % 2
        s = hg * 512
        m = {
            "xt": np.ascontiguousarray(x[b].T),
            "wq": np.ascontiguousarray(Wqkv[:, s:s + 512]),
            "wk": np.ascontiguousarray(Wqkv[:, C + s: C + s + 512]),
            "wv": np.ascontiguousarray(Wqkv[:, 2 * C + s: 2 * C + s + 512]),
            "wp": np.ascontiguousarray(Wproj[s:s + 512, :]),
            "bqc": np.ascontiguousarray(bqkv[s:s + 512].reshape(4, 128).T),
            "bkc": np.ascontiguousarray(bqkv[C + s: C + s + 512].reshape(4, 128).T),
            "bv": np.ascontiguousarray(bqkv[2 * C + s: 2 * C + s + 512][None, :]),
            "bp": np.ascontiguousarray(
                (bproj if hg == 0 else np.zeros_like(bproj))[None, :]),
            "ones_row": np.ones((1, 512), np.float32),
        }
        in_maps.append(m)
    return in_maps


def unshard_output(results):
    """Per-core partial outputs -> full [4, N, C]."""
    outs = []
    for b in range(4):
        outs.append(results[2 * b]["out"] + results[2 * b + 1]["out"])
    return np.stack(outs, axis=0)



# revision 2
# speedup vs baseline: 1.0970x; 1.0970x over previous
"""Self-contained Trainium2 Bass kernel: 16-head self-attention (B=4, N=2048,
C=1024, fp32), SPMD across 8 NeuronCores.

Entry point: kernel(**inputs) -> np.ndarray matching the reference module
(qkv projection + scaled-dot-product softmax attention + output projection).
See build_nc() docstring for the kernel design.
"""
import numpy as np

_NC_CACHE = {}


def kernel(x, Wqkv, bqkv, Wproj, bproj):
    from concourse.bass_utils import run_bass_kernel_spmd
    x = np.asarray(x, dtype=np.float32)
    Wqkv = np.asarray(Wqkv, dtype=np.float32)
    bqkv = np.asarray(bqkv, dtype=np.float32)
    Wproj = np.asarray(Wproj, dtype=np.float32)
    bproj = np.asarray(bproj, dtype=np.float32)
    # the rank-1 bias accumulation steps are emitted only when any bias is
    # actually nonzero (they are exact zeros in this problem's inputs)
    wb = bool(np.any(bqkv) or np.any(bproj))
    if wb not in _NC_CACHE:
        nc = build_nc(with_biases=wb)
        split_excess_waits(nc)
        _NC_CACHE[wb] = nc
    nc = _NC_CACHE[wb]
    in_maps = shard_inputs(x, Wqkv, bqkv, Wproj, bproj)
    res = run_bass_kernel_spmd(nc, in_maps, core_ids=list(range(N_CORES)))
    return unshard_output(res.results).astype(np.float32)


# ======================================================================
# IR post-pass: this walrus build accepts at most one semaphore wait per
# instruction; overflow waits move onto chained NoOps just before the
# instruction on the same engine queue.
# ======================================================================

# Walrus TPB_CTRL codegen (Drain/NoOp lowering) accepts only 1 sync wait;
# regular engine instructions accept more (tested empirically).
CTRL_OPCODES = {"Drain", "NoOp", "EventSemaphore", "AllEngineBarrier"}

def split_excess_waits(nc, engine_max=1, ctrl_max=1):
    n_split = 0
    for f in nc.m.functions:
        for bb in f.blocks:
            insts = list(bb.instructions)
            out = []
            changed = False
            for inst in insts:
                si = inst.sync_info
                max_w = ctrl_max if inst.opcode in CTRL_OPCODES else engine_max
                if si is not None and si.on_wait and len(si.on_wait) > max_w:
                    waits = list(si.on_wait)
                    extra, keep = waits[max_w:], waits[:max_w]
                    for i in range(0, len(extra), ctrl_max):
                        nop = bass_rust.InstNoOp(
                            name=f"{inst.name}-wsplit{i}", ins=[], outs=[])
                        nop.engine = inst.engine
                        nop.sync_info = mybir.SyncInfo(
                            on_wait=extra[i:i + ctrl_max], on_update=[])
                        out.append(nop)
                        n_split += 1
                    inst.sync_info = mybir.SyncInfo(
                        on_wait=keep, on_update=list(si.on_update))
                    changed = True
                out.append(inst)
            if changed:
                bb.instructions = out
    return n_split


# ======================================================================
# Kernel proper
# ======================================================================
import bass_rust
import concourse.bass as bass
import concourse.tile as tile
import concourse.mybir as mybir


F32 = mybir.dt.float32
BF16 = mybir.dt.bfloat16

N = 2048        # sequence length
C = 1024        # embed dim
HL = 8          # heads handled per core
D = 64          # head dim
SCALE = D ** -0.5
NHALF = N // 2
VS = D + 1      # v columns per head incl. ones column
N_CORES = 8

AFT = mybir.ActivationFunctionType
ALU = mybir.AluOpType


def build_nc(with_biases=True):
    MDT = BF16
    nc = bass.Bass("TRN2", target_bir_lowering=False, debug=False,
                   num_devices=N_CORES)
    xt = nc.dram_tensor("xt", [C, N], MDT, kind="ExternalInput").ap()
    wq = nc.dram_tensor("wq", [C, HL * D], MDT, kind="ExternalInput").ap()
    wk = nc.dram_tensor("wk", [C, HL * D], MDT, kind="ExternalInput").ap()
    wv = nc.dram_tensor("wv", [C, HL * D], MDT, kind="ExternalInput").ap()
    wp = nc.dram_tensor("wp", [HL * D, C], MDT, kind="ExternalInput").ap()
    bqc = nc.dram_tensor("bqc", [128, 4], F32, kind="ExternalInput").ap()
    bkc = nc.dram_tensor("bkc", [128, 4], F32, kind="ExternalInput").ap()
    bv = nc.dram_tensor("bv", [1, HL * D], MDT, kind="ExternalInput").ap()
    bp = nc.dram_tensor("bp", [1, C], MDT, kind="ExternalInput").ap()
    ones_row = nc.dram_tensor("ones_row", [1, 512], MDT, kind="ExternalInput").ap()
    out = nc.dram_tensor("out", [N, C], F32, kind="ExternalOutput").ap()

    with tile.TileContext(nc) as tc:
        with tc.tile_pool(name="consts", bufs=1) as consts, \
             tc.tile_pool(name="persist", bufs=1) as persist, \
             tc.tile_pool(name="big", bufs=1) as bigp:

            ones = consts.tile([1, 512], MDT, tag="ones")
            nc.sync.dma_start(out=ones, in_=ones_row)

            # persistent activation tiles (bf16 so the attention matmuls
            # get full 128x128 stationary tiles + fast weight loads):
            #   qT: pair-packed [2 heads' d x 128, pair-group x n]
            #   kTp: per-head [128, head x n] with the partner head's 64
            #        partition rows zeroed (K=128 contraction, zeros kill
            #        the partner-q contribution in the shared qT rhs)
            #   v_sb: per (m-chunk, head) [128, 65]: cols 0:64 v, col 64
            #        ones (softmax denominator)
            qT = persist.tile([128, 4 * N], BF16, tag="qT")
            kTp = persist.tile([128, HL * N], BF16, tag="kTp")
            v_sb = persist.tile([128, 16 * HL * VS], BF16, tag="v")
            nc.gpsimd.memset(kTp, 0.0)
            vview = v_sb.rearrange("p (m h e) -> p m h e", h=HL, e=VS)
            nc.gpsimd.memset(vview[:, :, :, D:D + 1], 1.0)

            # ---------- Phase A: xT, qT/kT, v ----------
            # xT in two half-tiles so the first matmul group only waits on
            # the first 2MB of x instead of the whole tensor.
            with tc.tile_pool(name="wpool", bufs=3) as wpool, \
                 tc.tile_pool(name="biases", bufs=1) as biasp, \
                 tc.tile_pool(name="xpool", bufs=2) as xpool, \
                 tc.tile_pool(name="kqp", bufs=2, space="PSUM") as kqp, \
                 tc.tile_pool(name="vpp", bufs=2, space="PSUM") as vpp:
                bqc_sb = biasp.tile([128, 4], F32, tag="bqc")
                bkc_sb = biasp.tile([128, 4], F32, tag="bkc")
                bv_sb = biasp.tile([1, HL * D], MDT, tag="bv")
                nc.sync.dma_start(out=bqc_sb, in_=bqc)
                nc.sync.dma_start(out=bkc_sb, in_=bkc)
                nc.sync.dma_start(out=bv_sb, in_=bv)
                # weight tiles: [128 (C-chunk partition), chunk x cols]
                wq_sb = wpool.tile([128, 8 * 512], MDT, tag="w", name="wq_sb")
                wk_sb = wpool.tile([128, 8 * 512], MDT, tag="w", name="wk_sb")
                wv_sb = wpool.tile([128, 8 * 512], MDT, tag="w", name="wv_sb")
                xTh = []
                for c in range(8):
                    nc.sync.dma_start(out=wq_sb[:, c * 512:(c + 1) * 512],
                                      in_=wq[c * 128:(c + 1) * 128, :])
                for nh in range(2):
                    xT = xpool.tile([128, 8 * NHALF], MDT, tag="x",
                                    name=f"xT{nh}")
                    xTh.append(xT)
                    for c in range(8):
                        nc.sync.dma_start(
                            out=xT[:, c * NHALF:(c + 1) * NHALF],
                            in_=xt[c * 128:(c + 1) * 128,
                                   nh * NHALF:(nh + 1) * NHALF])
                    if nh == 0:
                        for c in range(8):
                            nc.sync.dma_start(
                                out=wk_sb[:, c * 512:(c + 1) * 512],
                                in_=wk[c * 128:(c + 1) * 128, :])
                        for c in range(8):
                            nc.sync.dma_start(
                                out=wv_sb[:, c * 512:(c + 1) * 512],
                                in_=wv[c * 128:(c + 1) * 128, :])

                # qT / kT (bias folded into evacuation)
                for dstT, w_sb, b_col in ((qT, wq_sb, bqc_sb),
                                          (kTp, wk_sb, bkc_sb)):
                    for nh in range(2):
                        xT = xTh[nh]
                        for g in range(4):
                            for ngl in range(2):
                                ps = kqp.tile([128, 512], F32, tag="kq",
                                              name=f"kq{nh}_{g}_{ngl}")
                                for c in range(8):
                                    nc.tensor.matmul(
                                        ps,
                                        w_sb[:, c * 512 + g * 128:
                                             c * 512 + (g + 1) * 128],
                                        xT[:, c * NHALF + ngl * 512:
                                           c * NHALF + ngl * 512 + 512],
                                        start=(c == 0), stop=(c == 7))
                                n0 = nh * NHALF + ngl * 512
                                if dstT is qT:
                                    nc.vector.tensor_scalar_add(
                                        qT[:, g * N + n0: g * N + n0 + 512],
                                        ps, b_col[:, g:g + 1])
                                else:
                                    for hh in range(2):
                                        h_, r0_ = 2 * g + hh, hh * D
                                        nc.vector.tensor_scalar_add(
                                            kTp[r0_:r0_ + D,
                                                h_ * N + n0: h_ * N + n0 + 512],
                                            ps[r0_:r0_ + D, :],
                                            b_col[r0_:r0_ + D, g:g + 1])
                # v
                for nh in range(2):
                    xT = xTh[nh]
                    for ml in range(NHALF // 128):
                        mc = nh * (NHALF // 128) + ml
                        ps = vpp.tile([128, 512], F32, tag="v",
                                      name=f"v{nh}_{ml}")
                        for c in range(8):
                            nc.tensor.matmul(
                                ps,
                                xT[:, c * NHALF + ml * 128:
                                   c * NHALF + (ml + 1) * 128],
                                wv_sb[:, c * 512:(c + 1) * 512],
                                start=(c == 0),
                                stop=(c == 7 and not with_biases))
                        if with_biases:
                            nc.tensor.matmul(ps, ones[0:1, 0:128],
                                             bv_sb[0:1, :],
                                             start=False, stop=True)
                        dst = v_sb[:, mc * HL * VS:(mc + 1) * HL * VS].rearrange(
                            "p (h e) -> p h e", e=VS)[:, :, 0:D]
                        nc.vector.tensor_copy(
                            dst, ps.rearrange("p (h e) -> p h e", e=D))

            # attn_outT lives in its own tile (bf16, 16KB/partition total)
            aoT = bigp.tile([128, 4 * N], MDT, tag="big", name="aoT")

            # ---------- Phase B + C: attention with proj interleaved ----------
            with tc.tile_pool(name="wppool", bufs=1) as wppool, \
                 tc.tile_pool(name="expp", bufs=4) as expp, \
                 tc.tile_pool(name="avsp", bufs=2) as avsp, \
                 tc.tile_pool(name="denp", bufs=2) as denp, \
                 tc.tile_pool(name="bpp", bufs=1) as bpp, \
                 tc.tile_pool(name="pout", bufs=3) as pout:
                wp_sb = wppool.tile([128, 4 * C], MDT, tag="wp", name="wp_sb")
                for g in range(4):
                    nc.sync.dma_start(out=wp_sb[:, g * C:(g + 1) * C],
                                      in_=wp[g * 128:(g + 1) * 128, :])
                bp_sb = bpp.tile([1, C], MDT, tag="bp")
                nc.sync.dma_start(out=bp_sb, in_=bp)

                with tc.tile_pool(name="scp", bufs=2, space="PSUM") as scp, \
                     tc.tile_pool(name="avp", bufs=1, space="PSUM") as avp, \
                     tc.tile_pool(name="pjp", bufs=2, space="PSUM") as pjp:
                    # deferred normalization tails: each unit's divide chain
                    # is emitted after the NEXT unit's m-loop so its DMA/DVE
                    # latency never head-of-line-blocks the in-order PE queue
                    tails = []
                    # proj units for a finished half are injected between the
                    # next half's heads so the PE never drains at a boundary
                    proj_pending = []

                    def emit_proj(nch, jg):
                        ps = pjp.tile([128, 512], F32, tag="pj",
                                      name=f"pj{nch}_{jg}")
                        for g in range(4):
                            nc.tensor.matmul(
                                ps,
                                aoT[:, g * N + nch * 128:
                                    g * N + (nch + 1) * 128],
                                wp_sb[:, g * C + jg * 512:
                                      g * C + jg * 512 + 512],
                                start=(g == 0),
                                stop=(g == 3 and not with_biases))
                        if with_biases:
                            nc.tensor.matmul(
                                ps, ones[0:1, 0:128],
                                bp_sb[0:1, jg * 512:(jg + 1) * 512],
                                start=False, stop=True)
                        po = pout.tile([128, 512], F32, tag="po",
                                       name=f"po{nch}_{jg}")
                        nc.vector.tensor_copy(po, ps)
                        nc.sync.dma_start(
                            out=out[nch * 128:(nch + 1) * 128,
                                    jg * 512:(jg + 1) * 512],
                            in_=po)

                    for nh in range(2):
                        n0 = nh * NHALF
                        for h in range(HL):
                            g, r0 = h // 2, (h % 2) * D
                            av = avp.tile([VS, NHALF], F32, tag="av",
                                          name=f"av{h}_{nh}")

                            def av_acc(mcc, ex, av=av, h=h):
                                for ngl in range(2):
                                    nc.tensor.matmul(
                                        av[:, ngl * 512:(ngl + 1) * 512],
                                        v_sb[:, (mcc * HL + h) * VS:
                                             (mcc * HL + h + 1) * VS],
                                        ex[:, ngl * 512:(ngl + 1) * 512],
                                        start=(mcc == 0), stop=(mcc == 15))

                            # av lags sc/exp by 2 m-chunks so the in-order PE
                            # queue never stalls on an exp still in flight
                            pending = []
                            for mcc in range(16):
                                sc = scp.tile([128, NHALF], F32, tag="sc",
                                              name=f"sc{h}_{nh}_{mcc}")
                                for ngl in range(2):
                                    nc.tensor.matmul(
                                        sc[:, ngl * 512:(ngl + 1) * 512],
                                        kTp[:, h * N + mcc * 128:
                                            h * N + (mcc + 1) * 128],
                                        qT[:, g * N + n0 + ngl * 512:
                                           g * N + n0 + (ngl + 1) * 512],
                                        start=True, stop=True)
                                ex = expp.tile([128, NHALF], BF16, tag="ex",
                                               name=f"ex{h}_{nh}_{mcc}")
                                nc.scalar.activation(ex, sc, AFT.Exp,
                                                     scale=SCALE)
                                pending.append((mcc, ex))
                                if len(pending) > 2:
                                    av_acc(*pending.pop(0))
                            for it in pending:
                                av_acc(*it)
                            avs = avsp.tile([VS, NHALF], MDT, tag="avs",
                                            name=f"avs{h}_{nh}")
                            nc.vector.tensor_copy(avs, av[0:VS, :])
                            # reciprocal of the denominator row: DVE recip
                            # is ~6 cyc/elem along the free dim, so reshape
                            # [1,1024] -> [128,8] via sbuf-to-sbuf DMA, recip
                            # across 128 partitions, and reshape back
                            den = denp.tile([128, NHALF // 128], MDT,
                                            tag="den", name=f"den{h}_{nh}")
                            nc.sync.dma_start(out=den, in_=avs[D:VS, :])
                            rcp = denp.tile([128, NHALF // 128], MDT,
                                            tag="rcp", name=f"rcp{h}_{nh}")
                            with nc.allow_low_precision(reason="softmax denom"):
                                nc.vector.reciprocal(rcp, den)
                            rrow = denp.tile([1, NHALF], MDT, tag="rrow",
                                             name=f"rrow{h}_{nh}")
                            nc.sync.dma_start(out=rrow, in_=rcp)

                            def tail(h=h, nh=nh, n0=n0, g=g, r0=r0,
                                     avs=avs, rrow=rrow):
                                bc = scp.tile([D, NHALF], F32, tag="sc",
                                              name=f"bc{h}_{nh}")
                                for ngl in range(2):
                                    nc.tensor.matmul(
                                        bc[:, ngl * 512:(ngl + 1) * 512],
                                        ones[0:1, 0:D],
                                        rrow[0:1, ngl * 512:(ngl + 1) * 512],
                                        start=True, stop=True)
                                nc.vector.tensor_mul(
                                    aoT[r0:r0 + D,
                                        g * N + n0: g * N + n0 + NHALF],
                                    avs[0:D, :], bc)
                            tails.append(tail)
                            if len(tails) > 1:
                                tails.pop(0)()
                            # inject previous half's proj units (their aoT
                            # deps are long since satisfied)
                            if nh == 1 and h >= 1:
                                for _ in range(3):
                                    if proj_pending:
                                        proj_pending.pop(0)()
                        while tails:
                            tails.pop(0)()
                        for nl in range(8):
                            for jg in range(2):
                                proj_pending.append(
                                    lambda nch=nh * 8 + nl, jg=jg:
                                        emit_proj(nch, jg))
                    while proj_pending:
                        proj_pending.pop(0)()
    return nc


def _bf16(a):
    import ml_dtypes
    return np.ascontiguousarray(a).astype(ml_dtypes.bfloat16)


def shard_inputs(x, Wqkv, bqkv, Wproj, bproj):
    """Full inputs -> per-core in_maps. Core c: batch c//2, head-group c%2."""
    in_maps = []
    for core in range(N_CORES):
        b, hg = core // 2, core % 2
        s = hg * 512
        m = {
            "xt": _bf16(x[b].T),
            "wq": _bf16(Wqkv[:, s:s + 512]),
            "wk": _bf16(Wqkv[:, C + s: C + s + 512]),
            "wv": _bf16(Wqkv[:, 2 * C + s: 2 * C + s + 512]),
            "wp": _bf16(Wproj[s:s + 512, :]),
            "bqc": np.ascontiguousarray(bqkv[s:s + 512].reshape(4, 128).T),
            "bkc": np.ascontiguousarray(bqkv[C + s: C + s + 512].reshape(4, 128).T),
            "bv": _bf16(bqkv[2 * C + s: 2 * C + s + 512][None, :]),
            "bp": _bf16(
                (bproj if hg == 0 else np.zeros_like(bproj))[None, :]),
            "ones_row": _bf16(np.ones((1, 512), np.float32)),
        }
        in_maps.append(m)
    return in_maps


def unshard_output(results):
    """Per-core partial outputs -> full [4, N, C]."""
    outs = []
    for b in range(4):
        outs.append(results[2 * b]["out"] + results[2 * b + 1]["out"])
    return np.stack(outs, axis=0)
